# revision 18
# baseline (speedup 1.0000x reference)
"""Trainium2 Bass kernel for nn_AggregateStgcn (gnn_message_passing).

Computes, for x:(1,16,1,8192) f32, graph:(8192,8192) f32, fifo:(1,16,4,8192) f32,
stride=2:
    Asum[k, v] = sum_c x[0, c*4+k, 0, v]              (4, 8192)
    xsum[k, w] = sum_v Asum[k, v] * graph[v, w]       (4, 8192)
    S[k, w]    = sum_{j in 1,3,...,13} fifo[0, j, k, w]
    out[0, k, w, 0] = xsum[k, w] + S[k, w]            (1, 4, 8192, 1)

Sharding: graph is split column-wise across 8 NeuronCores (tensor parallel over
output nodes w); x is replicated; the fifo slice is local per core. No
collectives; host concatenates the 8 (4, 1024) output slices.

Precision/perf strategy: the kernel streams the whole graph once (1 byte per
element), and the PE streams every moving element once, so runtime is
max(HBM ~24us, PE-stream) plus ramp/tail overheads. Graph rows are split:
 - "normal" tiles: fp8 E3M4 (4 mantissa bits) at scale 256 against the bf16
   activation (A/256): 1 graph element/PE-cycle (216 ns / 128x512 tile-half);
 - DR tiles (stream tiles 32..55): fp8 E4M3 at scale 32 in DoubleRow perf
   mode: 2 graph elements/PE-cycle (HW-verified 216 ns per 256-row x 512-col
   pair). DoubleRow needs an fp8 stationary, so the activation rides as E4M3
   hi (cols 0:4, = A/32, products at scale 1) plus E4M3 lo of the residual
   x16 (cols 32:36, products 16x). DR accumulates into its own (48, 1024)
   PSUM region that closes before the e3m4 tail tiles, so the lo fold
   (dr_part = hi_acc + lo_acc/16, two ~1.2us DVE ops) hides under the tail
   matmuls; the final tail is two parallel half-adds (DVE + ACT) and two
   output DMAs.
Measured end-to-end error on the real inputs: ~1.3e-2 (max-err/max-expected)
vs the 2e-2 gate, deterministic (HW matmul is exact on the quantized values;
verified against host simulation).

Layout: the host pre-permutes the graph slice into the exact per-partition
stream order (for a chunk of s row-tiles starting at row off*128, partition p
holds rows off*128 + p*s + j), so every DMA is a plain 2D slice with one
long contiguous run per partition. A DoubleRow pair contracts two
consecutive stream tiles as k2=0/1 (contraction index = partition + 128*k2),
i.e. the moving AP is a (128, 2, 512) view of the chunk. x is shipped
pre-transposed and identically permuted as two (128, 32*16) bf16 halves
(pre-divided by 256) riding one DMA queue each, so the first half (and the
first real matmuls) are ready ~1.5us earlier; per half, three DVE adds
reduce the 16 channels to a (128, 32, 4) bf16 stationary. The DR region sits
entirely in the second half.

Schedule: ALL graph chunks are queued up front, each with its own SBUF
buffer (8.4 MB resident - no buffer-ring backpressure, DMA free-runs at
~360 GB/s). Tiny chunks first so the first matmuls start early; 512 KB
chunks alternate across the two HWDGE queues. Warmup matmuls (memset on
GPSIMD, whose queue opens first) open the PE HAM clock gate immediately;
filler matmuls bridge the first chunk-arrival gaps so the gate never drops
to half clock.
"""

import numpy as np

V = 8192
C = 4
K = 4
F = 16
NCORES = 8
WS = V // NCORES          # 1024 output columns per core
NT = V // 128             # 64 contraction tiles
NH = NT // 2              # tiles per x-half (32)
# chunk schedule: (kind, tiles). Regions in stream order:
#   head h: tiles 0..7   e3m4 (x-half A) - tiny chunks, ramp
#   DR   d: tiles 8..31  e4m3 DoubleRow (x-half A) - early, while the PE is
#            still DMA-gated; its PSUM region closes ~14us before the end
#   tail t: tiles 32..63 e3m4 (x-half B) - 1MB chunks at matched PE/DMA rate
CHUNK_PLAN = (
    [("h", 1), ("h", 1), ("h", 2), ("h", 2), ("h", 2)]
    + [("d", 8)] * 3
    + [("t", 8)] * 3 + [("t", 4), ("t", 2), ("t", 1), ("t", 1)]
)
CHUNKS = [s for _, s in CHUNK_PLAN]
NHEAD = sum(s for k, s in CHUNK_PLAN if k == "h")    # 8
NDR = sum(s for k, s in CHUNK_PLAN if k == "d")      # 24
NTAIL = sum(s for k, s in CHUNK_PLAN if k == "t")    # 32
NMID = 0
assert NHEAD + NDR == NH and NTAIL == NH
DR_LO = NHEAD                # 8: first DR tile
DR_HI = DR_LO + NDR          # 32
WARMUP_MM = 11               # throwaway matmuls to open the PE clock gate
                             # (must span a full 3.4us HAM window back-to-back)
FILLER_CHUNKS = 0            # no fillers: real matmuls start warm with a DMA lead
GSCALE = 256.0               # e3m4 graph pre-scale (x is pre-divided by this)
G4SCALE = 32.0               # e4m3 graph pre-scale for the DoubleRow region
DRW = 48                     # DoubleRow stationary packed width (16B-aligned)

TRACE = False                # set by test harness to capture an NTFF profile
LAST = None                  # BassKernelResults of the most recent run

_CACHED_NC = None


def _offs():
    return np.cumsum([0] + CHUNKS).tolist()


def _vmap():
    """vmap[t, p] = graph row held by partition p for contraction tile t."""
    offs = _offs()
    vm = np.empty((NT, 128), np.int64)
    for ci, s in enumerate(CHUNKS):
        off = offs[ci]
        for j in range(s):
            vm[off + j] = off * 128 + np.arange(128) * s + j
    return vm


def _build_nc():
    import concourse.bacc as bacc
    import concourse.mybir as mybir
    from concourse.tile import TileContext

    f32 = mybir.dt.float32
    bf16 = mybir.dt.bfloat16
    f8e3 = mybir.dt.float8e3
    f8e4 = mybir.dt.float8e4
    nc = bacc.Bacc(
        "TRN2",
        target_bir_lowering=False,
        debug=False,
        enable_asserts=False,
        num_devices=NCORES,
    )
    gh = nc.dram_tensor("gh", [128, NHEAD * WS], f8e3, kind="ExternalInput")
    gd = nc.dram_tensor("gd", [128, NDR * WS], f8e4, kind="ExternalInput")
    gt2 = nc.dram_tensor("gt2", [128, NTAIL * WS], f8e3, kind="ExternalInput")
    xta = nc.dram_tensor("xta", [128, NH * C * K], bf16, kind="ExternalInput")
    xtb = nc.dram_tensor("xtb", [128, NH * C * K], bf16, kind="ExternalInput")
    ffhi = nc.dram_tensor("ffhi", [7 * C, WS], bf16, kind="ExternalInput")
    selfm = nc.dram_tensor("selfm", [7 * C, K], bf16, kind="ExternalInput")
    out = nc.dram_tensor("out", [K, WS], f32, kind="ExternalOutput")

    offs = _offs()

    with TileContext(nc) as tc:
        with (
            tc.tile_pool(name="const", bufs=1) as cpool,
            tc.tile_pool(name="gp", bufs=1) as gpool,
            tc.tile_pool(name="ps", bufs=1, space="PSUM") as ppool,
        ):
            # PE warmup (outputs never read): the memset rides GPSIMD, whose
            # queue opens ~1.3us before DVE's, so the first warmup matmul
            # issues as early as possible and the HAM gate warms immediately.
            wtile = cpool.tile([128, 512], bf16)
            nc.gpsimd.memset(wtile[:], 1.0)
            wps = ppool.tile([128, 512], f32)

            def filler():
                nc.tensor.matmul(
                    wps[:], wtile[:, 0:128], wtile[:], start=True, stop=True
                )

            for _ in range(WARMUP_MM):
                filler()

            # small inputs first on both HWDGE queues, ahead of the graph;
            # the x halves ride one queue each
            xta_sb = cpool.tile([128, NH * C * K], bf16)
            nc.sync.dma_start(out=xta_sb[:], in_=xta.ap())
            selfm_sb = cpool.tile([7 * C, K], bf16)
            nc.scalar.dma_start(out=selfm_sb[:], in_=selfm.ap())
            ffhi_sb = cpool.tile([7 * C, WS], bf16)
            nc.scalar.dma_start(out=ffhi_sb[:], in_=ffhi.ap())
            xtb_sb = cpool.tile([128, NH * C * K], bf16)
            nc.scalar.dma_start(out=xtb_sb[:], in_=xtb.ap())

            # graph chunk DMAs: every chunk has its own resident buffer and
            # all transfers are queued up front (no ring backpressure)
            gts = []
            for ci, (kind, s) in enumerate(CHUNK_PLAN):
                off = offs[ci]
                if kind in ("h", "m"):
                    src = gh.ap()[:, off * WS : (off + s) * WS]
                    dt_ = f8e3
                elif kind == "d":
                    doff = off - DR_LO
                    src = gd.ap()[:, doff * WS : (doff + s) * WS]
                    dt_ = f8e4
                else:
                    toff = off - DR_HI
                    src = gt2.ap()[:, toff * WS : (toff + s) * WS]
                    dt_ = f8e3
                gt = gpool.tile([128, s * WS], dt_, name="gt", tag=f"gt{ci}")
                if ci % 2 == 0:
                    nc.sync.dma_start(out=gt[:], in_=src)
                else:
                    nc.scalar.dma_start(out=gt[:], in_=src)
                gts.append(gt)

            # DVE prep per x-half: reduce 16 channels to (128, 32, 4) bf16
            def prep_half(x_sb, sfx):
                xv = x_sb.rearrange("p (t a) -> p t a", a=C * K)
                u0 = cpool.tile([128, NH, K], bf16, name=f"u0{sfx}", tag=f"u0{sfx}")
                nc.vector.tensor_add(
                    out=u0[:], in0=xv[:, :, 0:K], in1=xv[:, :, K : 2 * K]
                )
                u1 = cpool.tile([128, NH, K], bf16, name=f"u1{sfx}", tag=f"u1{sfx}")
                nc.vector.tensor_add(
                    out=u1[:], in0=xv[:, :, 2 * K : 3 * K], in1=xv[:, :, 3 * K :]
                )
                a = cpool.tile([128, NH, K], bf16, name=f"a{sfx}", tag=f"a{sfx}")
                nc.vector.tensor_add(out=a[:], in0=u0[:], in1=u1[:])
                return a

            asumA = prep_half(xta_sb, "A")   # stream tiles 0..31

            # DoubleRow stationary from half A (tiles 8..31): hi =
            # e4m3(asum*8) = A/32 in cols 0:4, lo = e4m3((asum*8 - hi)*16)
            # in cols 32:36; other columns are junk (their product rows are
            # never read)
            asum_dr = cpool.tile([128, NDR, DRW], f8e4)
            a8 = cpool.tile([128, NDR, K], f32)
            nc.vector.tensor_scalar_mul(a8[:], asumA[:, DR_LO:DR_HI, :], 8.0)
            nc.vector.tensor_copy(out=asum_dr[:, :, 0:K], in_=a8[:])
            resid = cpool.tile([128, NDR, K], f32)
            nc.vector.tensor_sub(out=resid[:], in0=a8[:], in1=asum_dr[:, :, 0:K])
            nc.vector.tensor_scalar_mul(asum_dr[:, :, 32:36], resid[:], 16.0)

            asumB = prep_half(xtb_sb, "B")   # stream tiles 32..63

            # accumulators: (4, 1024) main (fifo + normal tiles, scale 1) and
            # (48, 1024) DR (rows 0:4 hi at scale 1, rows 32:36 lo at 16x)
            accM = ppool.tile([K, WS], f32, name="accM", tag="accM")
            accD = ppool.tile([DRW, WS], f32, name="accD", tag="accD")
            for h in range(2):
                hs = slice(h * 512, (h + 1) * 512)
                nc.tensor.matmul(
                    accM[:, hs], selfm_sb[:], ffhi_sb[:, hs],
                    start=True, stop=False,
                )

            def normal_chunk(ci, last_ci):
                s = CHUNKS[ci]
                off = offs[ci]
                gt = gts[ci]
                for j in range(s):
                    t = off + j
                    lastt = last_ci and j == s - 1
                    lhsT = (
                        asumA[:, t, :] if t < NH else asumB[:, t - NH, :]
                    )  # head tiles < 8 from A; tail tiles >= 32 from B
                    for h in range(2):
                        hs = slice(j * WS + h * 512, j * WS + (h + 1) * 512)
                        nc.tensor.matmul(
                            accM[:, h * 512 : (h + 1) * 512],
                            lhsT, gt[:, hs], start=False, stop=lastt,
                        )

            dr = mybir.MatmulPerfMode.DoubleRow
            kinds = [k for k, _ in CHUNK_PLAN]
            first_d = kinds.index("d")
            first_t = kinds.index("t")

            # head: tiny e3m4 chunks
            for ci in range(first_d):
                normal_chunk(ci, False)

            # DR chunks into accD (start on the first pair, stop on the
            # last - accD is finalized before the tail tiles)
            for ci in range(first_d, first_t):
                s = CHUNKS[ci]
                off = offs[ci]
                gtv = gts[ci].rearrange("p (j w) -> p j w", w=WS)
                for d in range(s // 2):
                    gdt = off - DR_LO + 2 * d
                    lhsT = asum_dr[:, gdt : gdt + 2, :]
                    for h in range(2):
                        nc.tensor.matmul(
                            accD[:, h * 512 : (h + 1) * 512],
                            lhsT,
                            gtv[:, 2 * d : 2 * d + 2, h * 512 : (h + 1) * 512],
                            start=(gdt == 0),
                            stop=(gdt == NDR - 2),
                            perf_mode=dr,
                        )

            # fold the DR region while the tail matmuls run:
            # dr_part = accD[0:4] + accD[32:36]/16
            lo_sb = cpool.tile([K, WS], f32)
            nc.vector.tensor_scalar_mul(lo_sb[:], accD[32:36, :], 1.0 / 16.0)
            dr_part = cpool.tile([K, WS], f32)
            nc.vector.tensor_add(out=dr_part[:], in0=lo_sb[:], in1=accD[0:K, :])

            # tail: e3m4 chunks; the last matmul closes the main group
            for ci in range(first_t, len(CHUNK_PLAN)):
                normal_chunk(ci, ci == len(CHUNK_PLAN) - 1)

            # final: out = dr_part + accM per half (DVE h=0, ACT cannot add
            # two tensors so DVE takes both adds; each half ships as soon as
            # it is ready)
            out_sb = cpool.tile([K, WS], f32)
            nc.vector.tensor_add(
                out=out_sb[:, 0:512], in0=dr_part[:, 0:512], in1=accM[:, 0:512]
            )
            nc.sync.dma_start(out=out.ap()[:, 0:512], in_=out_sb[:, 0:512])
            nc.vector.tensor_add(
                out=out_sb[:, 512:1024], in0=dr_part[:, 512:1024],
                in1=accM[:, 512:1024],
            )
            nc.scalar.dma_start(out=out.ap()[:, 512:1024], in_=out_sb[:, 512:1024])

    nc.compile()
    return nc


def kernel(x, graph, fifo, stride):
    global _CACHED_NC, LAST
    import ml_dtypes
    from concourse.bass_utils import run_bass_kernel_spmd

    bf16 = ml_dtypes.bfloat16
    e3m4 = ml_dtypes.float8_e3m4
    e4m3 = ml_dtypes.float8_e4m3
    x = np.asarray(x, dtype=np.float32)
    graph = np.asarray(graph, dtype=np.float32)
    fifo = np.asarray(fifo, dtype=np.float32)
    stride_v = int(np.asarray(stride))
    assert stride_v == 2, f"kernel hardcodes stride=2, got {stride_v}"

    vm = _vmap()                                  # (NT, 128)

    # graph rows permuted into stream order, per region
    rows_h = np.ascontiguousarray(vm[:DR_LO].T).reshape(-1)
    rows_d = np.ascontiguousarray(vm[DR_LO:DR_HI].T).reshape(-1)
    rows_t = np.ascontiguousarray(vm[DR_HI:].T).reshape(-1)
    qh = np.clip(graph[rows_h] * GSCALE, -15.5, 15.5).astype(e3m4)
    qd = np.clip(graph[rows_d] * G4SCALE, -240.0, 240.0).astype(e4m3)
    qt = np.clip(graph[rows_t] * GSCALE, -15.5, 15.5).astype(e3m4)

    def shard(q, ntile):
        qv = q.reshape(128, ntile, NCORES, WS)
        return [
            np.ascontiguousarray(qv[:, :, m]).reshape(128, ntile * WS)
            for m in range(NCORES)
        ]

    gh_sh = shard(qh, DR_LO)
    gd_sh = shard(qd, NDR)
    gt_sh = shard(qt, NTAIL)

    # x -> two (128, 32*16) bf16 halves, transposed + identically permuted,
    # pre-divided by the e3m4 graph scale
    xs = (x.reshape(C * K, V) * np.float32(1.0 / GSCALE)).astype(bf16)
    xtd = xs[:, vm.T].transpose(1, 2, 0).reshape(128, NT, C * K)
    xta = np.ascontiguousarray(xtd[:, :NH].reshape(128, NH * C * K))
    xtb = np.ascontiguousarray(xtd[:, NH:].reshape(128, NH * C * K))

    # odd fifo frames 1,3,...,13 -> per-core (28, 1024) bf16 slices
    ff_sh = np.ascontiguousarray(
        fifo.reshape(F, C, NCORES, WS)[1:14:2]
        .transpose(2, 0, 1, 3)
        .reshape(NCORES, 7 * C, WS)
    ).astype(bf16)
    eye = np.eye(K, dtype=np.float32)
    selfm = np.ascontiguousarray(np.tile(eye, (7, 1))).astype(bf16)

    if _CACHED_NC is None:
        _CACHED_NC = _build_nc()
    nc = _CACHED_NC

    in_maps = [
        {
            "gh": gh_sh[m], "gd": gd_sh[m], "gt2": gt_sh[m],
            "xta": xta, "xtb": xtb, "ffhi": ff_sh[m], "selfm": selfm,
        }
        for m in range(NCORES)
    ]
    res = run_bass_kernel_spmd(
        nc, in_maps, core_ids=list(range(NCORES)), trace=TRACE
    )
    LAST = res
    b = np.concatenate([res.results[m]["out"] for m in range(NCORES)], axis=1)
    return np.ascontiguousarray(b.reshape(1, C, V, 1))


# revision 19
# speedup vs baseline: 1.2162x; 1.2162x over previous
"""Trainium2 Bass kernel for nn_AggregateStgcn (gnn_message_passing).

Computes, for x:(1,16,1,8192) f32, graph:(8192,8192) f32, fifo:(1,16,4,8192) f32,
stride=2:
    Asum[k, v] = sum_c x[0, c*4+k, 0, v]              (4, 8192)
    xsum[k, w] = sum_v Asum[k, v] * graph[v, w]       (4, 8192)
    S[k, w]    = sum_{j in 1,3,...,13} fifo[0, j, k, w]
    out[0, k, w, 0] = xsum[k, w] + S[k, w]            (1, 4, 8192, 1)

Sharding: graph is split column-wise across 8 NeuronCores (tensor parallel over
output nodes w); x is replicated; the fifo slice is local per core. No
collectives; host concatenates the 8 (4, 1024) output slices.

Precision/perf strategy: the kernel streams the whole graph once (1 byte per
element), and the PE streams every moving element once, so runtime is
max(HBM ~24us, PE-stream) plus ramp/tail overheads. Graph rows are split:
 - "normal" tiles: fp8 E3M4 (4 mantissa bits) at scale 256 against the bf16
   activation (A/256): 1 graph element/PE-cycle (216 ns / 128x512 tile-half);
 - DR tiles (stream tiles 32..55): fp8 E4M3 at scale 32 in DoubleRow perf
   mode: 2 graph elements/PE-cycle (HW-verified 216 ns per 256-row x 512-col
   pair). DoubleRow needs an fp8 stationary, so the activation rides as E4M3
   hi (cols 0:4, = A/32, products at scale 1) plus E4M3 lo of the residual
   x16 (cols 32:36, products 16x). DR accumulates into its own (48, 1024)
   PSUM region that closes before the e3m4 tail tiles, so the lo fold
   (dr_part = hi_acc + lo_acc/16, two ~1.2us DVE ops) hides under the tail
   matmuls; the final tail is two parallel half-adds (DVE + ACT) and two
   output DMAs.
Measured end-to-end error on the real inputs: ~1.3e-2 (max-err/max-expected)
vs the 2e-2 gate, deterministic (HW matmul is exact on the quantized values;
verified against host simulation).

Layout: the host pre-permutes the graph slice into the exact per-partition
stream order (for a chunk of s row-tiles starting at row off*128, partition p
holds rows off*128 + p*s + j), so every DMA is a plain 2D slice with one
long contiguous run per partition. A DoubleRow pair contracts two
consecutive stream tiles as k2=0/1 (contraction index = partition + 128*k2),
i.e. the moving AP is a (128, 2, 512) view of the chunk. x is shipped
pre-transposed and identically permuted as two (128, 32*16) bf16 halves
(pre-divided by 256) riding one DMA queue each, so the first half (and the
first real matmuls) are ready ~1.5us earlier; per half, three DVE adds
reduce the 16 channels to a (128, 32, 4) bf16 stationary. The DR region sits
entirely in the second half.

Schedule: ALL graph chunks are queued up front, each with its own SBUF
buffer (8.4 MB resident - no buffer-ring backpressure, DMA free-runs at
~360 GB/s). Tiny chunks first so the first matmuls start early; 512 KB
chunks alternate across the two HWDGE queues. Warmup matmuls (memset on
GPSIMD, whose queue opens first) open the PE HAM clock gate immediately;
filler matmuls bridge the first chunk-arrival gaps so the gate never drops
to half clock.
"""

import numpy as np

V = 8192
C = 4
K = 4
F = 16
NCORES = 8
WS = V // NCORES          # 1024 output columns per core
NT = V // 128             # 64 contraction tiles
NH = NT // 2              # tiles per x-half (32)
# chunk schedule: (kind, tiles). Regions in stream order:
#   head h: tiles 0..5   e3m4 (x-half A) - tiny chunks, ramp
#   DR   d: tiles 6..29  e4m3 DoubleRow (x-half A) - early: a cold-clock DR
#            matmul (427ns per 2 tiles) still matches the ~300GB/s DMA rate,
#            so the HAM ramp costs nothing here; the DR PSUM region closes
#            ~15us before the end so its fold hides completely
#   tail t: tiles 30..63 e3m4 (30..31 x-half A, rest x-half B) - 512KB
#            chunks; the PE is warm and tracks the DMA at matched rate
CHUNK_PLAN = (
    [("h", 1), ("h", 1), ("h", 2), ("h", 2)]
    + [("d", 4)] * 5 + [("d", 2)] * 2
    + [("t", 4)] * 8 + [("t", 2)]
)
CHUNKS = [s for _, s in CHUNK_PLAN]
NHEAD = sum(s for k, s in CHUNK_PLAN if k == "h")    # 6
NDR = sum(s for k, s in CHUNK_PLAN if k == "d")      # 24
NTAIL = sum(s for k, s in CHUNK_PLAN if k == "t")    # 34
NMID = 0
assert NHEAD + NDR + NTAIL == NT
DR_LO = NHEAD                # 6: first DR tile
DR_HI = DR_LO + NDR          # 30
WARMUP_MM = 6                # throwaway matmuls to open the PE clock gate
FILLER_CHUNKS = 3            # head chunks 1..N followed by 2 fillers each
GSCALE = 256.0               # e3m4 graph pre-scale (x is pre-divided by this)
G4SCALE = 32.0               # e4m3 graph pre-scale for the DoubleRow region
DRW = 48                     # DoubleRow stationary packed width (16B-aligned)

TRACE = False                # set by test harness to capture an NTFF profile
LAST = None                  # BassKernelResults of the most recent run

_CACHED_NC = None


def _offs():
    return np.cumsum([0] + CHUNKS).tolist()


def _vmap():
    """vmap[t, p] = graph row held by partition p for contraction tile t."""
    offs = _offs()
    vm = np.empty((NT, 128), np.int64)
    for ci, s in enumerate(CHUNKS):
        off = offs[ci]
        for j in range(s):
            vm[off + j] = off * 128 + np.arange(128) * s + j
    return vm


def _build_nc():
    import concourse.bacc as bacc
    import concourse.mybir as mybir
    from concourse.tile import TileContext

    f32 = mybir.dt.float32
    bf16 = mybir.dt.bfloat16
    f8e3 = mybir.dt.float8e3
    f8e4 = mybir.dt.float8e4
    nc = bacc.Bacc(
        "TRN2",
        target_bir_lowering=False,
        debug=False,
        enable_asserts=False,
        num_devices=NCORES,
    )
    gh = nc.dram_tensor("gh", [128, NHEAD * WS], f8e3, kind="ExternalInput")
    gd = nc.dram_tensor("gd", [128, NDR * WS], f8e4, kind="ExternalInput")
    gt2 = nc.dram_tensor("gt2", [128, NTAIL * WS], f8e3, kind="ExternalInput")
    xta = nc.dram_tensor("xta", [128, NH * C * K], bf16, kind="ExternalInput")
    xtb = nc.dram_tensor("xtb", [128, NH * C * K], bf16, kind="ExternalInput")
    ffhi = nc.dram_tensor("ffhi", [7 * C, WS], bf16, kind="ExternalInput")
    selfm = nc.dram_tensor("selfm", [7 * C, K], bf16, kind="ExternalInput")
    out = nc.dram_tensor("out", [K, WS], f32, kind="ExternalOutput")

    offs = _offs()

    with TileContext(nc) as tc:
        with (
            tc.tile_pool(name="const", bufs=1) as cpool,
            tc.tile_pool(name="gp", bufs=1) as gpool,
            tc.tile_pool(name="ps", bufs=1, space="PSUM") as ppool,
        ):
            # PE warmup (outputs never read): the memset rides GPSIMD, whose
            # queue opens ~1.3us before DVE's, so the first warmup matmul
            # issues as early as possible and the HAM gate warms immediately.
            wtile = cpool.tile([128, 512], bf16)
            nc.gpsimd.memset(wtile[:], 1.0)
            wps = ppool.tile([128, 512], f32)

            def filler():
                nc.tensor.matmul(
                    wps[:], wtile[:, 0:128], wtile[:], start=True, stop=True
                )

            for _ in range(WARMUP_MM):
                filler()

            # small inputs first on both HWDGE queues, ahead of the graph;
            # the x halves ride one queue each
            xta_sb = cpool.tile([128, NH * C * K], bf16)
            nc.sync.dma_start(out=xta_sb[:], in_=xta.ap())
            selfm_sb = cpool.tile([7 * C, K], bf16)
            nc.scalar.dma_start(out=selfm_sb[:], in_=selfm.ap())
            ffhi_sb = cpool.tile([7 * C, WS], bf16)
            nc.scalar.dma_start(out=ffhi_sb[:], in_=ffhi.ap())
            xtb_sb = cpool.tile([128, NH * C * K], bf16)
            nc.scalar.dma_start(out=xtb_sb[:], in_=xtb.ap())

            # graph chunk DMAs: every chunk has its own resident buffer and
            # all transfers are queued up front (no ring backpressure)
            gts = []
            for ci, (kind, s) in enumerate(CHUNK_PLAN):
                off = offs[ci]
                if kind in ("h", "m"):
                    src = gh.ap()[:, off * WS : (off + s) * WS]
                    dt_ = f8e3
                elif kind == "d":
                    doff = off - DR_LO
                    src = gd.ap()[:, doff * WS : (doff + s) * WS]
                    dt_ = f8e4
                else:
                    toff = off - DR_HI
                    src = gt2.ap()[:, toff * WS : (toff + s) * WS]
                    dt_ = f8e3
                gt = gpool.tile([128, s * WS], dt_, name="gt", tag=f"gt{ci}")
                if ci % 2 == 0:
                    nc.sync.dma_start(out=gt[:], in_=src)
                else:
                    nc.scalar.dma_start(out=gt[:], in_=src)
                gts.append(gt)

            # DVE prep per x-half: reduce 16 channels to (128, 32, 4) bf16
            def prep_half(x_sb, sfx):
                xv = x_sb.rearrange("p (t a) -> p t a", a=C * K)
                u0 = cpool.tile([128, NH, K], bf16, name=f"u0{sfx}", tag=f"u0{sfx}")
                nc.vector.tensor_add(
                    out=u0[:], in0=xv[:, :, 0:K], in1=xv[:, :, K : 2 * K]
                )
                u1 = cpool.tile([128, NH, K], bf16, name=f"u1{sfx}", tag=f"u1{sfx}")
                nc.vector.tensor_add(
                    out=u1[:], in0=xv[:, :, 2 * K : 3 * K], in1=xv[:, :, 3 * K :]
                )
                a = cpool.tile([128, NH, K], bf16, name=f"a{sfx}", tag=f"a{sfx}")
                nc.vector.tensor_add(out=a[:], in0=u0[:], in1=u1[:])
                return a

            asumA = prep_half(xta_sb, "A")   # stream tiles 0..31

            # DoubleRow stationary from half A (tiles 8..31): hi =
            # e4m3(asum*8) = A/32 in cols 0:4, lo = e4m3((asum*8 - hi)*16)
            # in cols 32:36; other columns are junk (their product rows are
            # never read)
            asum_dr = cpool.tile([128, NDR, DRW], f8e4)
            a8 = cpool.tile([128, NDR, K], f32)
            nc.vector.tensor_scalar_mul(a8[:], asumA[:, DR_LO:DR_HI, :], 8.0)
            nc.vector.tensor_copy(out=asum_dr[:, :, 0:K], in_=a8[:])
            resid = cpool.tile([128, NDR, K], f32)
            nc.vector.tensor_sub(out=resid[:], in0=a8[:], in1=asum_dr[:, :, 0:K])
            nc.vector.tensor_scalar_mul(asum_dr[:, :, 32:36], resid[:], 16.0)

            asumB = prep_half(xtb_sb, "B")   # stream tiles 32..63

            # accumulators: (4, 1024) main (fifo + normal tiles, scale 1) and
            # (48, 1024) DR (rows 0:4 hi at scale 1, rows 32:36 lo at 16x)
            accM = ppool.tile([K, WS], f32, name="accM", tag="accM")
            accD = ppool.tile([DRW, WS], f32, name="accD", tag="accD")
            for h in range(2):
                hs = slice(h * 512, (h + 1) * 512)
                nc.tensor.matmul(
                    accM[:, hs], selfm_sb[:], ffhi_sb[:, hs],
                    start=True, stop=False,
                )

            def normal_chunk(ci, last_ci):
                s = CHUNKS[ci]
                off = offs[ci]
                gt = gts[ci]
                for j in range(s):
                    t = off + j
                    lastt = last_ci and j == s - 1
                    lhsT = (
                        asumA[:, t, :] if t < NH else asumB[:, t - NH, :]
                    )  # head tiles < 8 from A; tail tiles >= 32 from B
                    for h in range(2):
                        hs = slice(j * WS + h * 512, j * WS + (h + 1) * 512)
                        nc.tensor.matmul(
                            accM[:, h * 512 : (h + 1) * 512],
                            lhsT, gt[:, hs], start=False, stop=lastt,
                        )

            dr = mybir.MatmulPerfMode.DoubleRow
            kinds = [k for k, _ in CHUNK_PLAN]
            first_d = kinds.index("d")
            first_t = kinds.index("t")

            # head: tiny e3m4 chunks; fillers bridge early DMA gaps
            for ci in range(first_d):
                normal_chunk(ci, False)
                if 1 <= ci <= FILLER_CHUNKS:
                    filler()
                    filler()

            # DR chunks into accD (start on the first pair, stop on the
            # last - accD is finalized before the tail tiles)
            for ci in range(first_d, first_t):
                s = CHUNKS[ci]
                off = offs[ci]
                gtv = gts[ci].rearrange("p (j w) -> p j w", w=WS)
                for d in range(s // 2):
                    gdt = off - DR_LO + 2 * d
                    lhsT = asum_dr[:, gdt : gdt + 2, :]
                    for h in range(2):
                        nc.tensor.matmul(
                            accD[:, h * 512 : (h + 1) * 512],
                            lhsT,
                            gtv[:, 2 * d : 2 * d + 2, h * 512 : (h + 1) * 512],
                            start=(gdt == 0),
                            stop=(gdt == NDR - 2),
                            perf_mode=dr,
                        )

            # fold the DR region while the tail matmuls run:
            # dr_part = accD[0:4] + accD[32:36]/16
            lo_sb = cpool.tile([K, WS], f32)
            nc.vector.tensor_scalar_mul(lo_sb[:], accD[32:36, :], 1.0 / 16.0)
            dr_part = cpool.tile([K, WS], f32)
            nc.vector.tensor_add(out=dr_part[:], in0=lo_sb[:], in1=accD[0:K, :])

            # tail: e3m4 chunks; the last matmul closes the main group
            for ci in range(first_t, len(CHUNK_PLAN)):
                normal_chunk(ci, ci == len(CHUNK_PLAN) - 1)

            # final: out = dr_part + accM per half (DVE h=0, ACT cannot add
            # two tensors so DVE takes both adds; each half ships as soon as
            # it is ready)
            out_sb = cpool.tile([K, WS], f32)
            nc.vector.tensor_add(
                out=out_sb[:, 0:512], in0=dr_part[:, 0:512], in1=accM[:, 0:512]
            )
            nc.sync.dma_start(out=out.ap()[:, 0:512], in_=out_sb[:, 0:512])
            nc.vector.tensor_add(
                out=out_sb[:, 512:1024], in0=dr_part[:, 512:1024],
                in1=accM[:, 512:1024],
            )
            nc.scalar.dma_start(out=out.ap()[:, 512:1024], in_=out_sb[:, 512:1024])

    nc.compile()
    return nc


def kernel(x, graph, fifo, stride):
    global _CACHED_NC, LAST
    import ml_dtypes
    from concourse.bass_utils import run_bass_kernel_spmd

    bf16 = ml_dtypes.bfloat16
    e3m4 = ml_dtypes.float8_e3m4
    e4m3 = ml_dtypes.float8_e4m3
    x = np.asarray(x, dtype=np.float32)
    graph = np.asarray(graph, dtype=np.float32)
    fifo = np.asarray(fifo, dtype=np.float32)
    stride_v = int(np.asarray(stride))
    assert stride_v == 2, f"kernel hardcodes stride=2, got {stride_v}"

    vm = _vmap()                                  # (NT, 128)

    # graph rows permuted into stream order, per region
    rows_h = np.ascontiguousarray(vm[:DR_LO].T).reshape(-1)
    rows_d = np.ascontiguousarray(vm[DR_LO:DR_HI].T).reshape(-1)
    rows_t = np.ascontiguousarray(vm[DR_HI:].T).reshape(-1)
    qh = np.clip(graph[rows_h] * GSCALE, -15.5, 15.5).astype(e3m4)
    qd = np.clip(graph[rows_d] * G4SCALE, -240.0, 240.0).astype(e4m3)
    qt = np.clip(graph[rows_t] * GSCALE, -15.5, 15.5).astype(e3m4)

    def shard(q, ntile):
        qv = q.reshape(128, ntile, NCORES, WS)
        return [
            np.ascontiguousarray(qv[:, :, m]).reshape(128, ntile * WS)
            for m in range(NCORES)
        ]

    gh_sh = shard(qh, DR_LO)
    gd_sh = shard(qd, NDR)
    gt_sh = shard(qt, NTAIL)

    # x -> two (128, 32*16) bf16 halves, transposed + identically permuted,
    # pre-divided by the e3m4 graph scale
    xs = (x.reshape(C * K, V) * np.float32(1.0 / GSCALE)).astype(bf16)
    xtd = xs[:, vm.T].transpose(1, 2, 0).reshape(128, NT, C * K)
    xta = np.ascontiguousarray(xtd[:, :NH].reshape(128, NH * C * K))
    xtb = np.ascontiguousarray(xtd[:, NH:].reshape(128, NH * C * K))

    # odd fifo frames 1,3,...,13 -> per-core (28, 1024) bf16 slices
    ff_sh = np.ascontiguousarray(
        fifo.reshape(F, C, NCORES, WS)[1:14:2]
        .transpose(2, 0, 1, 3)
        .reshape(NCORES, 7 * C, WS)
    ).astype(bf16)
    eye = np.eye(K, dtype=np.float32)
    selfm = np.ascontiguousarray(np.tile(eye, (7, 1))).astype(bf16)

    if _CACHED_NC is None:
        _CACHED_NC = _build_nc()
    nc = _CACHED_NC

    in_maps = [
        {
            "gh": gh_sh[m], "gd": gd_sh[m], "gt2": gt_sh[m],
            "xta": xta, "xtb": xtb, "ffhi": ff_sh[m], "selfm": selfm,
        }
        for m in range(NCORES)
    ]
    res = run_bass_kernel_spmd(
        nc, in_maps, core_ids=list(range(NCORES)), trace=TRACE
    )
    LAST = res
    b = np.concatenate([res.results[m]["out"] for m in range(NCORES)], axis=1)
    return np.ascontiguousarray(b.reshape(1, C, V, 1))
